# revision 27
# baseline (speedup 1.0000x reference)
"""Trainium2 Bass kernel for nn_Attention_light_dwconv_v3.

Data-parallel over batch: 32 batches -> 8 cores x 4 batches. No collectives.

Per-batch on-core pipeline (all layouts transposed: channels on partitions):
  x [3136,320] f32 --SWDGE cast DMA--> x_bf16 DRAM --xbar transpose DMA-->
  xT bf16 [320,3136] in SBUF (3 chunks of <=128 channels)
  - dwconv 4x4/s4: 16 strided taps, per-partition scalar MAC on DVE
  - pointwise 320->400 matmul (fp32), bias via ones-row; LN stats via
    ones-vector matmuls (channel dim is on partitions); gelu on ACT
  - kT [80,196]/head, v_aug [196, 5*(64+1)] (ones col per head => softmax
    denominator rides the attn@v matmul for free)
  - qT [80,3136]/head = q_w^T @ xT
  - S^T [196, n] = kT^T-matmul, exp on ACT (scale folded, no max-shift:
    scores are O(1) here, fp32 exp is safe, softmax is shift-invariant)
  - att_outT = (v_aug^T @ attnT) with row 64 = denominator; eviction does
    the divide (TT divide, PSUM -> SBUF bf16)
  - proj: att_outT chunks as lhsT, proj bias via ones-row, y f32 out
"""

import os
import sys
from contextlib import ExitStack

import numpy as np

sys.path.insert(0, "/opt/trn_rl_repo")

import ml_dtypes

import concourse.bass as bass
import concourse.mybir as mybir
from concourse import bacc
from concourse.alu_op_type import AluOpType
from concourse.bass_utils import run_bass_kernel_spmd
from concourse.tile import TileContext

BF16 = mybir.dt.bfloat16
F32 = mybir.dt.float32
AF = mybir.ActivationFunctionType

B, N, C = 32, 3136, 320
CN, HEADS, DQ, DV = 400, 5, 80, 64
NK = 196  # (56/4)^2
SR = 4
SCALE = (C // HEADS * 1.25) ** -0.5  # 80^-0.5
NCORES = 8
BL = B // NCORES  # batches per core

# K-chunks over C=320 matching the three transpose-DMA'd xT tensors:
# xT0 = ch 0:128, xT1 = ch 128:256, xT2 = ch 256:320 (from padded scratch,
# rows 64:128 of xT2 are junk). All operands base-partition 0.
QCHUNKS = [(0, 128), (1, 128), (2, 64)]  # (xT idx, rows)
DWCHUNKS = [(0, 128), (1, 128), (2, 64)]

# CN=400 chunks for k/v matmul contraction and pw output M-tiles
CNCH = [(0, 128), (128, 128), (256, 128), (384, 16)]

# n tiles (free dim of attention/q matmuls)
NT = [(i * 512, min(512, N - i * 512)) for i in range((N + 511) // 512)]
# proj/output token tiles
MT = [(i * 128, min(128, N - i * 128)) for i in range((N + 127) // 128)]
# k-token chunks
KT = [(0, 128), (128, 68)]

_built = None


def build_kernel():
    nc = bacc.Bacc("TRN2", target_bir_lowering=False)

    x_in = nc.dram_tensor("x", [BL, N, C], F32, kind="ExternalInput")
    y_out = nc.dram_tensor("y", [BL, N, C], F32, kind="ExternalOutput")

    w_specs = {
        "qw0": ([128, CN], BF16), "qw1": ([128, CN], BF16), "qw2": ([64, CN], BF16),
        "kw0": ([128, CN], BF16), "kw1": ([128, CN], BF16),
        "kw2": ([128, CN], BF16), "kw3": ([16, CN], BF16),
        "vw0": ([128, C], BF16), "vw1": ([128, C], BF16),
        "vw2": ([128, C], BF16), "vw3": ([16, C], BF16),
        "pwt0": ([128, CN], F32), "pwt1": ([128, CN], F32), "pwt2": ([65, CN], F32),
        "prw0": ([128, C], BF16), "prw1": ([128, C], BF16), "prw2": ([65, C], BF16),
        "dwf0": ([128, 16], F32), "dwf1": ([128, 16], F32), "dwf2": ([64, 16], F32),
        "lng": ([128, 4], F32), "lnb": ([128, 4], F32),
    }
    w_dram = {k: nc.dram_tensor(k, sh, dt, kind="ExternalInput")
              for k, (sh, dt) in w_specs.items()}

    with TileContext(nc) as tc, ExitStack() as ctx:
        cpool = ctx.enter_context(tc.tile_pool(name="consts", bufs=1))
        dram = ctx.enter_context(tc.tile_pool(name="dram", bufs=1, space="DRAM"))
        xt_pool = ctx.enter_context(tc.tile_pool(name="xt", bufs=2))
        sp_pool = ctx.enter_context(tc.tile_pool(name="spatial", bufs=2))
        q_pool = ctx.enter_context(tc.tile_pool(name="qt", bufs=1))
        at_pool = ctx.enter_context(tc.tile_pool(name="attnT", bufs=2))
        ao_pool = ctx.enter_context(tc.tile_pool(name="attout", bufs=1))
        y_pool = ctx.enter_context(tc.tile_pool(name="ysb", bufs=3))
        ps_a = ctx.enter_context(tc.tile_pool(name="ps_a", bufs=4, space="PSUM"))
        ps_b = ctx.enter_context(tc.tile_pool(name="ps_b", bufs=4, space="PSUM"))

        # ---- load weights ----
        w = {}
        for k, (sh, dt) in w_specs.items():
            w[k] = cpool.tile(sh, dt, tag=k, name=k)
            nc.sync.dma_start(out=w[k], in_=w_dram[k][:, :])

        ones_sb = cpool.tile([128, 1], F32, tag="ones")
        nc.vector.memset(ones_sb, 1.0)
        ones_row = cpool.tile([1, 128], F32, tag="ones_row")
        nc.vector.memset(ones_row, 1.0)

        xbf = dram.tile([BL, N, C], BF16)
        # padded scratch for channels 256:320 so the third transpose chunk
        # has a 128-wide source (cols 64:128 stay junk, never read)
        xbf2 = dram.tile([BL, N, 128], BF16)

        qw = [w["qw0"], w["qw1"], w["qw2"]]
        kw = [w["kw0"], w["kw1"], w["kw2"], w["kw3"]]
        vw = [w["vw0"], w["vw1"], w["vw2"], w["vw3"]]
        pwt = [w["pwt0"], w["pwt1"], w["pwt2"]]
        prw = [w["prw0"], w["prw1"], w["prw2"]]
        dwf = [w["dwf0"], w["dwf1"], w["dwf2"]]

        for b in range(BL):
            # ---- x -> bf16 (cast DMA), then xbar-transpose into SBUF ----
            nc.gpsimd.dma_start(out=xbf[b], in_=x_in[b])
            nc.gpsimd.dma_start(out=xbf2[b, :, 0:64], in_=x_in[b, :, 256:320])
            # fill the pad half with valid data too (never read by compute,
            # but keeps the transpose-DMA source fully initialized)
            nc.gpsimd.dma_start(out=xbf2[b, :, 64:128], in_=x_in[b, :, 256:320])
            xT = []
            for k in range(3):
                t = xt_pool.tile([128, N], BF16, tag=f"xt{k}")
                if k < 2:
                    nc.sync.dma_start_transpose(out=t, in_=xbf[b, :, 128 * k:128 * (k + 1)])
                else:
                    nc.sync.dma_start_transpose(out=t, in_=xbf2[b, :, :])
                xT.append(t)

            # ---- spatial reduction: dwconv (DVE MACs) ----
            # acc[:, c, :] = chunk c of dwconv output [ch, 196] f32
            acc = sp_pool.tile([128, 3, NK], F32, tag="acc")
            for ci, (xi, rows) in enumerate(DWCHUNKS):
                xr = xT[xi].rearrange("p (ri a sj b) -> p ri a sj b",
                                      ri=14, a=SR, sj=14, b=SR)
                o = acc[0:rows, ci, :].rearrange("p (ri sj) -> p ri sj", sj=14)
                for tap in range(16):
                    di, dj = tap // SR, tap % SR
                    sl = xr[0:rows, :, di, :, dj]
                    sc = dwf[ci][0:rows, tap:tap + 1]
                    if tap == 0:
                        nc.vector.tensor_scalar_mul(o, sl, sc)
                    else:
                        nc.vector.scalar_tensor_tensor(
                            out=o, in0=sl, scalar=sc, in1=o,
                            op0=AluOpType.mult, op1=AluOpType.add)
            # ones row for pw bias (acc chunk2 row 64)
            nc.vector.memset(acc[64:65, 2, :], 1.0)

            # ---- pointwise conv 320->400 (+bias), fp32 matmul ----
            xs_pre = sp_pool.tile([128, 4, NK], F32, tag="xs_pre")
            xs_sq = sp_pool.tile([128, 4, NK], F32, tag="xs_sq")
            xsg = sp_pool.tile([128, 4, NK], BF16, tag="xsg")
            for m, (m0, ms) in enumerate(CNCH):
                pxs = ps_a.tile([128, NK], F32, tag="mm512")
                nc.tensor.matmul(pxs[0:ms, :], pwt[0][:, m0:m0 + ms],
                                 acc[0:128, 0, :], start=True, stop=False)
                nc.tensor.matmul(pxs[0:ms, :], pwt[1][:, m0:m0 + ms],
                                 acc[0:128, 1, :], start=False, stop=False)
                nc.tensor.matmul(pxs[0:ms, :], pwt[2][0:65, m0:m0 + ms],
                                 acc[0:65, 2, :], start=False, stop=True)
                nc.vector.tensor_copy(out=xs_pre[0:ms, m, :], in_=pxs[0:ms, :])
                nc.scalar.activation(out=xs_sq[0:ms, m, :], in_=pxs[0:ms, :],
                                     func=AF.Square)

            # ---- layernorm over 400 channels (on partitions) ----
            psum = ps_b.tile([1, NK], F32, tag="mmsmall")
            psq = ps_b.tile([1, NK], F32, tag="mmsmall")
            for m, (m0, ms) in enumerate(CNCH):
                nc.tensor.matmul(psum[0:1, :], ones_sb[0:ms, 0:1],
                                 xs_pre[0:ms, m, :], start=(m == 0), stop=(m == 3))
            for m, (m0, ms) in enumerate(CNCH):
                nc.tensor.matmul(psq[0:1, :], ones_sb[0:ms, 0:1],
                                 xs_sq[0:ms, m, :], start=(m == 0), stop=(m == 3))
            # mr[0,0,:] = mean, mr[0,1,:] = rstd -- then broadcast to 128
            # partitions with a K=1 ones-matmul (DVE cannot partition-bcast)
            mr = sp_pool.tile([1, 2, NK], F32, tag="mr")
            vv = sp_pool.tile([1, NK], F32, tag="vv")
            tmp = sp_pool.tile([1, NK], F32, tag="tmp")
            nc.vector.tensor_scalar_mul(mr[0:1, 0, :], psum, 1.0 / CN)
            nc.vector.tensor_scalar_mul(vv, psq, 1.0 / CN)
            # vv = E[x^2] - mu^2 + eps
            nc.vector.tensor_tensor(out=tmp, in0=mr[0:1, 0, :], in1=mr[0:1, 0, :],
                                    op=AluOpType.mult)
            nc.vector.tensor_tensor(out=vv, in0=vv, in1=tmp, op=AluOpType.subtract)
            nc.vector.tensor_scalar_add(vv, vv, 1e-5)
            # rstd = rsqrt(vv) via mult-only Newton, seed min(1/vv, 2.5).
            # (ACT Sqrt lives in a different act-table set than Exp/Tanh;
            # avoiding it avoids ~2.7us table reloads per use.)
            y = mr[0:1, 1, :]
            nc.vector.reciprocal(out=y, in_=vv)
            nc.vector.tensor_scalar_min(y, y, 2.5)
            for _ in range(9):
                nc.vector.tensor_tensor(out=tmp, in0=y, in1=y, op=AluOpType.mult)
                nc.vector.tensor_tensor(out=tmp, in0=tmp, in1=vv, op=AluOpType.mult)
                nc.vector.tensor_scalar(out=tmp, in0=tmp, scalar1=-0.5,
                                        scalar2=1.5, op0=AluOpType.mult,
                                        op1=AluOpType.add)
                nc.vector.tensor_tensor(out=y, in0=y, in1=tmp, op=AluOpType.mult)
            pmr = ps_b.tile([128, 2, NK], F32, tag="mmsmall")
            nc.tensor.matmul(pmr[:, :, :].rearrange("p a b -> p (a b)"),
                             ones_row[0:1, 0:128],
                             mr[0:1, :, :].rearrange("p a b -> p (a b)"),
                             start=True, stop=True)

            # normalize + gelu(tanh approx, stays in the Exp act-table set)
            C0, C1 = 0.7978845608028654, 0.044715
            for m, (m0, ms) in enumerate(CNCH):
                t = sp_pool.tile([128, NK], F32, tag="normt")
                s = sp_pool.tile([128, NK], F32, tag="sqt")
                nc.vector.tensor_tensor(out=t[0:ms, :], in0=xs_pre[0:ms, m, :],
                                        in1=pmr[0:ms, 0, :],
                                        op=AluOpType.subtract)
                nc.vector.tensor_tensor(out=t[0:ms, :], in0=t[0:ms, :],
                                        in1=pmr[0:ms, 1, :],
                                        op=AluOpType.mult)
                nc.vector.tensor_scalar(out=t[0:ms, :], in0=t[0:ms, :],
                                        scalar1=w["lng"][0:ms, m:m + 1],
                                        scalar2=w["lnb"][0:ms, m:m + 1],
                                        op0=AluOpType.mult, op1=AluOpType.add)
                # gelu(t) = 0.5*t*(1 + tanh(C0*(t + C1*t^3)))
                nc.scalar.activation(out=s[0:ms, :], in_=t[0:ms, :], func=AF.Square)
                nc.vector.tensor_scalar(out=s[0:ms, :], in0=s[0:ms, :],
                                        scalar1=C1, scalar2=1.0,
                                        op0=AluOpType.mult, op1=AluOpType.add)
                nc.vector.tensor_tensor(out=s[0:ms, :], in0=s[0:ms, :],
                                        in1=t[0:ms, :], op=AluOpType.mult)
                nc.scalar.activation(out=s[0:ms, :], in_=s[0:ms, :], func=AF.Tanh,
                                     scale=C0)
                nc.vector.tensor_scalar(out=s[0:ms, :], in0=s[0:ms, :],
                                        scalar1=0.5, scalar2=0.5,
                                        op0=AluOpType.mult, op1=AluOpType.add)
                nc.vector.tensor_tensor(out=xsg[0:ms, m, :], in0=s[0:ms, :],
                                        in1=t[0:ms, :], op=AluOpType.mult)

            # ---- kT [80, 196] per head ----
            kT = sp_pool.tile([80, HEADS, NK], BF16, tag="kT")
            for h in range(HEADS):
                pk = ps_b.tile([80, NK], F32, tag="mmsmall")
                for m, (m0, ms) in enumerate(CNCH):
                    nc.tensor.matmul(pk[:, :], kw[m][0:ms, DQ * h:DQ * (h + 1)],
                                     xsg[0:ms, m, :], start=(m == 0), stop=(m == 3))
                nc.vector.tensor_copy(out=kT[:, h, :], in_=pk[:, :])

            # ---- v_aug [196, 5*128]: per-head [v (64 cols) | ones (64 cols)]
            # The 64 ones-columns replicate the softmax denominator across
            # partitions 64:128 of the attn@v PSUM tile, so the normalizing
            # divide is a plain 64-partition DVE op.
            v_aug = sp_pool.tile([128, 2, 5 * 128], BF16, tag="vaug")
            for ti, (t0, tsz) in enumerate(KT):
                pv = ps_b.tile([128, C], F32, tag="mmsmall")
                for m, (m0, ms) in enumerate(CNCH):
                    nc.tensor.matmul(pv[0:tsz, :], xsg[0:ms, m, t0:t0 + tsz],
                                     vw[m][0:ms, :], start=(m == 0), stop=(m == 3))
                for h in range(HEADS):
                    nc.vector.tensor_copy(
                        out=v_aug[0:tsz, ti, h * 128:h * 128 + 64],
                        in_=pv[0:tsz, DV * h:DV * (h + 1)])
                nc.vector.memset(
                    v_aug[0:tsz, ti, :].rearrange("p (h e) -> p h e", e=128)[:, :, 64:128],
                    1.0)

            # ---- qT [80, 3136] per head ----
            qT = q_pool.tile([80, HEADS, N], BF16, tag="qT")
            for h in range(HEADS):
                for (nt0, ntw) in NT:
                    pq = ps_a.tile([80, 512], F32, tag="mm512")
                    for (xi, rows) in QCHUNKS:
                        nc.tensor.matmul(
                            pq[:, 0:ntw],
                            qw[xi][0:rows, DQ * h:DQ * (h + 1)],
                            xT[xi][0:rows, nt0:nt0 + ntw],
                            start=(xi == 0), stop=(xi == 2))
                    nc.scalar.copy(out=qT[:, h, nt0:nt0 + ntw], in_=pq[:, 0:ntw])

            # ---- attention per head ----
            att_outT = ao_pool.tile([128, 3, N], BF16, tag="aoT")
            nc.vector.memset(att_outT[64:65, 2, :], 1.0)  # proj bias ones-row
            for h in range(HEADS):
                a0 = at_pool.tile([128, N], BF16, tag="a0")
                a1 = at_pool.tile([68, N], BF16, tag="a1")
                for (nt0, ntw) in NT:
                    ps0 = ps_a.tile([128, 512], F32, tag="mm512")
                    ps1 = ps_a.tile([68, 512], F32, tag="mm512")
                    nc.tensor.matmul(ps0[:, 0:ntw], kT[:, h, 0:128],
                                     qT[:, h, nt0:nt0 + ntw], start=True, stop=True)
                    nc.tensor.matmul(ps1[:, 0:ntw], kT[:, h, 128:NK],
                                     qT[:, h, nt0:nt0 + ntw], start=True, stop=True)
                    nc.scalar.activation(out=a0[:, nt0:nt0 + ntw], in_=ps0[:, 0:ntw],
                                         func=AF.Exp, scale=SCALE)
                    nc.scalar.activation(out=a1[:, nt0:nt0 + ntw], in_=ps1[:, 0:ntw],
                                         func=AF.Exp, scale=SCALE)
                    pav = ps_b.tile([128, 512], F32, tag="mmsmall")
                    nc.tensor.matmul(pav[:, 0:ntw],
                                     v_aug[0:128, 0, h * 128:h * 128 + 128],
                                     a0[:, nt0:nt0 + ntw], start=True, stop=False)
                    nc.tensor.matmul(pav[:, 0:ntw],
                                     v_aug[0:68, 1, h * 128:h * 128 + 128],
                                     a1[0:68, nt0:nt0 + ntw], start=False, stop=True)
                    # evict + normalize: att_out = av * (1/denom)
                    # (pav rows 0:64 = attn@v, rows 64:128 = denominator
                    # copies). DVE has no TT divide, and reciprocal_approx's
                    # bit-trick misbehaves reading PSUM (HW-measured 13% err),
                    # so: ACT copies denom to SBUF, DVE inverts (~18-bit,
                    # 1 custom op), DVE TT-mult normalizes at eviction.
                    den = y_pool.tile([64, 512], F32, tag="den")
                    rec = y_pool.tile([64, 512], F32, tag="rec")
                    nc.scalar.copy(out=den[:, 0:ntw], in_=pav[64:128, 0:ntw])
                    nc.vector.reciprocal_approx_fast(out=rec[:, 0:ntw],
                                                     in_=den[:, 0:ntw])
                    nc.vector.tensor_tensor(
                        out=att_outT[64 * (h % 2):64 * (h % 2) + 64, h // 2,
                                     nt0:nt0 + ntw],
                        in0=pav[0:64, 0:ntw],
                        in1=rec[:, 0:ntw],
                        op=AluOpType.mult)

            # ---- proj (+bias via ones-row) ----
            for (m0, ms) in MT:
                py = ps_b.tile([128, C], F32, tag="mmsmall")
                nc.tensor.matmul(py[0:ms, :], att_outT[0:128, 0, m0:m0 + ms],
                                 prw[0][:, :], start=True, stop=False)
                nc.tensor.matmul(py[0:ms, :], att_outT[0:128, 1, m0:m0 + ms],
                                 prw[1][:, :], start=False, stop=False)
                nc.tensor.matmul(py[0:ms, :], att_outT[0:65, 2, m0:m0 + ms],
                                 prw[2][0:65, :], start=False, stop=True)
                ysb = y_pool.tile([128, C], F32, tag="ysb")
                nc.scalar.copy(out=ysb[0:ms, :], in_=py[0:ms, :])
                nc.sync.dma_start(out=y_out[b, m0:m0 + ms, :], in_=ysb[0:ms, :])

    nc.finalize()
    return nc


def _prep_weights(dw_w, dw_b, pw_w, pw_b, ln_g, ln_b, q_w, k_w, v_w,
                  proj_w, proj_b):
    bf = ml_dtypes.bfloat16
    f = np.float32
    dw_w, dw_b = np.asarray(dw_w, f), np.asarray(dw_b, f)
    pw_w, pw_b = np.asarray(pw_w, f), np.asarray(pw_b, f)
    ln_g, ln_b = np.asarray(ln_g, f), np.asarray(ln_b, f)
    q_w, k_w, v_w = np.asarray(q_w, f), np.asarray(k_w, f), np.asarray(v_w, f)
    proj_w, proj_b = np.asarray(proj_w, f), np.asarray(proj_b, f)

    out = {}
    out["qw0"] = q_w[0:128].astype(bf)
    out["qw1"] = q_w[128:256].astype(bf)
    out["qw2"] = q_w[256:320].astype(bf)
    for i, (r0, rs) in enumerate(CNCH):
        out[f"kw{i}"] = k_w[r0:r0 + rs].astype(bf)
        out[f"vw{i}"] = v_w[r0:r0 + rs].astype(bf)
    pwt = np.ascontiguousarray(pw_w.T)  # [320, 400]
    pw_b_eff = pw_b + pw_w @ dw_b
    out["pwt0"] = pwt[0:128].astype(f)
    out["pwt1"] = pwt[128:256].astype(f)
    out["pwt2"] = np.concatenate([pwt[256:320], pw_b_eff[None, :]], 0).astype(f)
    out["prw0"] = proj_w[0:128].astype(bf)
    out["prw1"] = proj_w[128:256].astype(bf)
    out["prw2"] = np.concatenate([proj_w[256:320], proj_b[None, :]], 0).astype(bf)
    dwf = dw_w.reshape(C, 16)
    out["dwf0"] = dwf[0:128].astype(f)
    out["dwf1"] = dwf[128:256].astype(f)
    out["dwf2"] = dwf[256:320].astype(f)
    lng = np.zeros((128, 4), f)
    lnb = np.zeros((128, 4), f)
    for m, (m0, ms) in enumerate(CNCH):
        lng[0:ms, m] = ln_g[m0:m0 + ms]
        lnb[0:ms, m] = ln_b[m0:m0 + ms]
    out["lng"], out["lnb"] = lng, lnb
    return out


LAST_RESULT = None


def kernel(x, H, W, dw_w, dw_b, pw_w, pw_b, ln_g, ln_b, q_w, k_w, v_w,
           proj_w, proj_b):
    global _built, LAST_RESULT
    assert int(H) == 56 and int(W) == 56
    x = np.asarray(x, np.float32)
    assert x.shape == (B, N, C), x.shape

    if _built is None:
        _built = build_kernel()
    nc = _built

    wmaps = _prep_weights(dw_w, dw_b, pw_w, pw_b, ln_g, ln_b, q_w, k_w, v_w,
                          proj_w, proj_b)
    in_maps = []
    for c in range(NCORES):
        m = {"x": np.ascontiguousarray(x[c * BL:(c + 1) * BL])}
        m.update(wmaps)
        in_maps.append(m)

    trace = os.environ.get("KERNEL_TRACE", "0") == "1"
    res = run_bass_kernel_spmd(nc, in_maps, core_ids=list(range(NCORES)),
                               trace=trace)
    LAST_RESULT = res
    y = np.concatenate([r["y"] for r in res.results], axis=0)
    return y.astype(np.float32)


if __name__ == "__main__":
    rng = np.random.default_rng(0)
    print("smoke test: building kernel IR only")
    nc = build_kernel()
    print("built OK:", len(nc.m.functions[0].instructions)
          if hasattr(nc.m.functions[0], "instructions") else "n/a")
